# revision 8
# baseline (speedup 1.0000x reference)
"""Multi-head causal attention (B=4, L=2048, D=1024, H=16) on 8 trn2 cores.

Sharding: (batch, head-group) grid — core c handles batch c//2, heads
(c%2)*8..(c%2)*8+8.  Each core projects Q/K/V for its 8 heads, runs causal
attention, and computes a partial output projection; the host sums the two
head-group partials per batch.

Per-core layouts (host prepares transposed inputs so every matmul contracts
over the partition dim):
  xq_t/xk_t/xv_t [D, L]   : x.T            (rhs / lhsT of projections)
  wq_t/wk_t/wv_t [D, 512] : W_slice.T      (wq pre-scaled by 1/sqrt(dh))
  wo_t           [512, D] : Wo_slice.T
  qT/kT pair tiles [128, L]: rows 0-63 head 2p, 64-127 head 2p+1 (dh on P)
  v_aug [128, 8, 65]      : per 128-token chunk; [:, h, 0:64]=V, [:, h, 64]=key mask
  scores ST [k(P), q(F)]  : transposed scores -> softmax sum via matmul's
                            extra mask column (pv row 64), no P-transposes.
"""

import math
from contextlib import ExitStack

import numpy as np

import concourse.bass as bass
import concourse.tile as tile
from concourse import bacc, mybir
from concourse import bass_utils

D = 1024  # model dim
HG = 512  # head dims per core (8 heads x 64)
NH = 8    # heads per core
DH = 64
NPAIR = 4  # head pairs per core
NEG = -1.0e30

F32 = mybir.dt.float32
BF16 = mybir.dt.bfloat16
EXP = mybir.ActivationFunctionType.Exp


def build(L=2048):
    TQ = L // 512    # 512-token q-blocks
    T16 = L // 128   # 128-token chunks
    DCH = D // 128   # contraction chunks for projections

    nc = bacc.Bacc("TRN2", target_bir_lowering=False, debug=False, num_devices=8)

    xq = nc.dram_tensor("xq_t", [D, L], BF16, kind="ExternalInput").ap()
    xk = nc.dram_tensor("xk_t", [D, L], BF16, kind="ExternalInput").ap()
    xv = nc.dram_tensor("xv_t", [D, L], BF16, kind="ExternalInput").ap()
    wq = nc.dram_tensor("wq_t", [D, HG], BF16, kind="ExternalInput").ap()
    wk = nc.dram_tensor("wk_t", [D, HG], BF16, kind="ExternalInput").ap()
    wv = nc.dram_tensor("wv_t", [D, HG], BF16, kind="ExternalInput").ap()
    wo = nc.dram_tensor("wo_t", [HG, D], BF16, kind="ExternalInput").ap()
    mcol = nc.dram_tensor("maskcol", [L, NH], BF16, kind="ExternalInput").ap()
    trim = nc.dram_tensor("trimask", [128, 128], F32, kind="ExternalInput").ap()
    outp = nc.dram_tensor("outp", [L, D], F32, kind="ExternalOutput").ap()

    with ExitStack() as ctx:
        tc = ctx.enter_context(tile.TileContext(nc))

        # ---- persistent tiles ----
        singles = ctx.enter_context(tc.tile_pool(name="singles", bufs=1))
        qT = [singles.tile([128, L], BF16, tag=f"qT{p}", name=f"qT{p}") for p in range(NPAIR)]
        kT = [singles.tile([128, L], BF16, tag=f"kT{p}", name=f"kT{p}") for p in range(NPAIR)]
        vaug = [singles.tile([128, NH, DH + 1], BF16, tag=f"vaug{t}", name=f"vaug{t}") for t in range(T16)]
        ctxT = [singles.tile([128, L], BF16, tag=f"ctxT{p}", name=f"ctxT{p}") for p in range(NPAIR)]
        mc_sb = singles.tile([128, T16, NH], BF16, tag="mc")
        tri_sb = singles.tile([128, 128], F32, tag="tri")
        ones_sb = singles.tile([128, DH], F32, tag="ones")

        nc.sync.dma_start(out=mc_sb, in_=mcol.rearrange("(t p) h -> p t h", p=128))
        nc.sync.dma_start(out=tri_sb, in_=trim)
        nc.vector.memset(ones_sb, 1.0)

        # =========== Phase 1: projections ===========
        with (
            tc.tile_pool(name="xt", bufs=10) as xtp,
            tc.tile_pool(name="w", bufs=3 * DCH) as wp,
            tc.tile_pool(name="pproj", bufs=3, space="PSUM") as ppp,
        ):
            w_sb = {}
            for name, wdram in (("k", wk), ("v", wv), ("q", wq)):
                w_sb[name] = [wp.tile([128, HG], BF16, tag="w", name="wtile") for _ in range(DCH)]
                for d in range(DCH):
                    nc.sync.dma_start(out=w_sb[name][d], in_=wdram[d * 128:(d + 1) * 128, :])

            def proj_transposed(xdram, wtiles, dst):
                # dst[m][:, t*512:+512] = (W.T chunk m).T @ xT  -> [outdim(P), tok(F)]
                for t in range(TQ):
                    xts = [xtp.tile([128, 512], BF16, tag="xt", name="xtile") for _ in range(DCH)]
                    for d in range(DCH):
                        nc.sync.dma_start(
                            out=xts[d],
                            in_=xdram[d * 128:(d + 1) * 128, t * 512:(t + 1) * 512],
                        )
                    for m in range(NPAIR):
                        ps = ppp.tile([128, 512], F32, tag="pproj")
                        for d in range(DCH):
                            nc.tensor.matmul(
                                ps,
                                lhsT=(wtiles[d][:, m * 128:(m + 1) * 128]),
                                rhs=(xts[d]),
                                start=(d == 0),
                                stop=(d == DCH - 1),
                            )
                        nc.vector.tensor_copy(dst[m][:, t * 512:(t + 1) * 512], ps)

            def proj_natural_v(xdram, wtiles):
                # v_aug[t16][:, h, 0:64] = (x @ Wv.T)[tok chunk, head h]
                for t in range(TQ):
                    xts = [xtp.tile([128, 512], BF16, tag="xt", name="xtile") for _ in range(DCH)]
                    for d in range(DCH):
                        nc.sync.dma_start(
                            out=xts[d],
                            in_=xdram[d * 128:(d + 1) * 128, t * 512:(t + 1) * 512],
                        )
                    for s in range(4):  # 128-token subchunks
                        t16 = t * 4 + s
                        ps = ppp.tile([128, 512], F32, tag="pproj")
                        for d in range(DCH):
                            nc.tensor.matmul(
                                ps,
                                lhsT=(xts[d][:, s * 128:(s + 1) * 128]),
                                rhs=(wtiles[d]),
                                start=(d == 0),
                                stop=(d == DCH - 1),
                            )
                        nc.vector.tensor_copy(
                            vaug[t16][:, :, 0:DH],
                            ps.rearrange("p (h e) -> p h e", h=NH),
                        )
                        nc.vector.tensor_copy(
                            vaug[t16][:, :, DH:DH + 1],
                            mc_sb[:, t16:t16 + 1, :],
                        )

            proj_transposed(xk, w_sb["k"], kT)
            proj_natural_v(xv, w_sb["v"])
            proj_transposed(xq, w_sb["q"], qT)

        # =========== Phase 2: attention ===========
        with (
            tc.tile_pool(name="stp", bufs=2, space="PSUM") as stp,
            tc.tile_pool(name="pvp", bufs=2, space="PSUM") as pvp,
            tc.tile_pool(name="bcp", bufs=1, space="PSUM") as bcp,
            tc.tile_pool(name="expp", bufs=4) as expp,
            tc.tile_pool(name="recp", bufs=2) as recp,
            tc.tile_pool(name="bcsb", bufs=2) as bcsbp,
            tc.tile_pool(name="tmpb", bufs=2) as tmpbp,
        ):
            for p in range(NPAIR):
                for qb in range(TQ):
                    nkc = 4 * (qb + 1)  # causal k-chunks for this q-block
                    pv = [pvp.tile([DH + 1, 512], F32, tag="pv", name="pvtile") for _ in range(2)]
                    for kc in range(nkc):
                        j = kc - 4 * qb  # >=0 -> diagonal 512-block
                        off = j * 128 if j >= 0 else 0
                        st = stp.tile([128, 2, 512], F32, tag="st")
                        for ph in range(2):
                            nc.tensor.matmul(
                                st[:, ph, off:512],
                                lhsT=(kT[p][ph * DH:(ph + 1) * DH,
                                              kc * 128:(kc + 1) * 128]),
                                rhs=(qT[p][ph * DH:(ph + 1) * DH,
                                             qb * 512 + off:(qb + 1) * 512]),
                                start=True,
                                stop=True,
                            )
                        if j >= 0:  # causal mask on the first 128 q-columns
                            for ph in range(2):
                                nc.vector.tensor_add(
                                    st[:, ph, off:off + 128],
                                    st[:, ph, off:off + 128],
                                    tri_sb,
                                )
                        ex = expp.tile([128, 2, 512], BF16, tag="expst")
                        nc.scalar.activation(
                            out=ex[:, :, off:512], in_=st[:, :, off:512], func=EXP
                        )
                        for ph in range(2):
                            nc.tensor.matmul(
                                pv[ph][:, off:512],
                                lhsT=(vaug[kc][:, 2 * p + ph, :]),
                                rhs=(ex[:, ph, off:512]),
                                start=(kc == 0),
                                stop=(kc == nkc - 1),
                            )
                    # normalize: ctxT[p][:, qb block] = pv[0:64] / pv[64]
                    for ph in range(2):
                        rec = recp.tile([128, 512], F32, tag="rec")
                        nc.vector.reciprocal(rec[DH:DH + 1, :], pv[ph][DH:DH + 1, :])
                        bc = bcp.tile([DH, 512], F32, tag="bc")
                        nc.tensor.matmul(
                            bc,
                            lhsT=(ones_sb[DH:DH + 1, :]),
                            rhs=(rec[DH:DH + 1, :]),
                            start=True,
                            stop=True,
                        )
                        bcs = bcsbp.tile([DH, 512], F32, tag="bcs")
                        nc.vector.tensor_copy(bcs, bc)
                        if ph == 0:
                            nc.vector.tensor_mul(
                                ctxT[p][0:DH, qb * 512:(qb + 1) * 512],
                                pv[ph][0:DH, :],
                                bcs,
                            )
                        else:
                            tb = tmpbp.tile([DH, 512], BF16, tag="tb")
                            nc.vector.tensor_mul(tb, pv[ph][0:DH, :], bcs)
                            # partition shift rows 0-63 -> 64-127 via DMA
                            nc.sync.dma_start(
                                out=ctxT[p][DH:128, qb * 512:(qb + 1) * 512], in_=tb
                            )

        # =========== Phase 3: output projection (partial) ===========
        with (
            tc.tile_pool(name="wop", bufs=NPAIR) as wop,
            tc.tile_pool(name="outp_sb", bufs=3) as outsb,
            tc.tile_pool(name="po", bufs=3, space="PSUM") as pop,
        ):
            wo_sb = [wop.tile([128, D], BF16, tag="wo", name="wotile") for _ in range(NPAIR)]
            for c in range(NPAIR):
                nc.sync.dma_start(out=wo_sb[c], in_=wo[c * 128:(c + 1) * 128, :])
            for t16 in range(T16):
                ot = outsb.tile([128, D], F32, tag="ot")
                for oh in range(2):
                    ps = pop.tile([128, 512], F32, tag="po")
                    for c in range(NPAIR):
                        nc.tensor.matmul(
                            ps,
                            lhsT=(ctxT[c][:, t16 * 128:(t16 + 1) * 128]),
                            rhs=(wo_sb[c][:, oh * 512:(oh + 1) * 512]),
                            start=(c == 0),
                            stop=(c == NPAIR - 1),
                        )
                    nc.vector.tensor_copy(ot[:, oh * 512:(oh + 1) * 512], ps)
                nc.sync.dma_start(out=outp[t16 * 128:(t16 + 1) * 128, :], in_=ot)

    nc.compile()
    return nc


_CACHE = {}


def _get_nc(L):
    if L not in _CACHE:
        _CACHE[L] = build(L)
    return _CACHE[L]


def make_in_maps(query, key, value, attention_mask, Wq, Wk, Wv, Wo):
    import ml_dtypes

    B, L, _ = query.shape
    scale = np.float32(1.0 / math.sqrt(DH))
    bf = lambda a: np.ascontiguousarray(np.asarray(a, np.float32)).astype(
        ml_dtypes.bfloat16
    )
    xqT = [bf(np.asarray(query[b]).T) for b in range(B)]
    xkT = [bf(np.asarray(key[b]).T) for b in range(B)]
    xvT = [bf(np.asarray(value[b]).T) for b in range(B)]
    kk, qq = np.meshgrid(np.arange(128), np.arange(128), indexing="ij")
    tri = np.ascontiguousarray(
        np.where(kk > qq, np.float32(NEG), np.float32(0.0)).astype(np.float32)
    )
    in_maps = []
    for core in range(2 * B):
        b, hg = divmod(core, 2)
        sl = slice(hg * HG, (hg + 1) * HG)
        mc = np.repeat(np.asarray(attention_mask[b]).astype(np.float32)[:, None], NH, 1)
        in_maps.append({
            "xq_t": xqT[b],
            "xk_t": xkT[b],
            "xv_t": xvT[b],
            "wq_t": bf(np.asarray(Wq, np.float32)[sl, :].T * scale),
            "wk_t": bf(np.asarray(Wk, np.float32)[sl, :].T),
            "wv_t": bf(np.asarray(Wv, np.float32)[sl, :].T),
            "wo_t": bf(np.asarray(Wo, np.float32)[:, sl].T),
            "maskcol": bf(mc),
            "trimask": tri,
        })
    return in_maps


def kernel(query, key, value, attention_mask, Wq, Wk, Wv, Wo, _res_hook=None):
    B, L, D_ = query.shape
    nc = _get_nc(L)
    in_maps = make_in_maps(query, key, value, attention_mask, Wq, Wk, Wv, Wo)
    res = bass_utils.run_bass_kernel_spmd(nc, in_maps, core_ids=list(range(8)))
    if _res_hook is not None:
        _res_hook(res)
    out = np.empty((B, L, D_), np.float32)
    for b in range(B):
        out[b] = res.results[2 * b]["outp"] + res.results[2 * b + 1]["outp"]
    return out


# revision 13
# speedup vs baseline: 1.2359x; 1.2359x over previous
"""Multi-head causal attention (B=4, L=2048, D=1024, H=16) on 8 trn2 cores.

Sharding: (batch, head-group) grid — core c handles batch c//2, heads
(c%2)*8..(c%2)*8+8.  Each core projects Q/K/V for its 8 heads, runs causal
attention, and computes a partial output projection; the host sums the two
head-group partials per batch.

Per-core layouts (host prepares transposed inputs so every matmul contracts
over the partition dim):
  xq_t/xk_t/xv_t [D, L]   : x.T            (rhs / lhsT of projections)
  wq_t/wk_t/wv_t [D, 512] : W_slice.T      (wq pre-scaled by 1/sqrt(dh))
  wo_t           [512, D] : Wo_slice.T
  qT/kT pair tiles [128, L]: rows 0-63 head 2p, 64-127 head 2p+1 (dh on P)
  v_aug [128, 8, 65]      : per 128-token chunk; [:, h, 0:64]=V, [:, h, 64]=key mask
  scores ST [k(P), q(F)]  : transposed scores -> softmax sum via matmul's
                            extra mask column (pv row 64), no P-transposes.
"""

import math
from contextlib import ExitStack

import numpy as np

import concourse.bass as bass
import concourse.tile as tile
from concourse import bacc, mybir
from concourse import bass_utils

D = 1024  # model dim
HG = 512  # head dims per core (8 heads x 64)
NH = 8    # heads per core
DH = 64
NPAIR = 4  # head pairs per core
NEG = -1.0e30

F32 = mybir.dt.float32
BF16 = mybir.dt.bfloat16
EXP = mybir.ActivationFunctionType.Exp


def build(L=2048):
    TQ = L // 512    # 512-token q-blocks
    T16 = L // 128   # 128-token chunks
    DCH = D // 128   # contraction chunks for projections

    nc = bacc.Bacc("TRN2", target_bir_lowering=False, debug=False, num_devices=8)

    xq = nc.dram_tensor("xq_t", [D, L], BF16, kind="ExternalInput").ap()
    xk = nc.dram_tensor("xk_t", [D, L], BF16, kind="ExternalInput").ap()
    xv = nc.dram_tensor("xv_t", [D, L], BF16, kind="ExternalInput").ap()
    wq = nc.dram_tensor("wq_t", [D, HG], BF16, kind="ExternalInput").ap()
    wk = nc.dram_tensor("wk_t", [D, HG], BF16, kind="ExternalInput").ap()
    wv = nc.dram_tensor("wv_t", [D, HG], BF16, kind="ExternalInput").ap()
    wo = nc.dram_tensor("wo_t", [HG, D], BF16, kind="ExternalInput").ap()
    mcol = nc.dram_tensor("maskcol", [L, NH], BF16, kind="ExternalInput").ap()
    trim = nc.dram_tensor("trimask", [128, 128], F32, kind="ExternalInput").ap()
    outp = nc.dram_tensor("outp", [L, D], F32, kind="ExternalOutput").ap()

    with ExitStack() as ctx:
        tc = ctx.enter_context(tile.TileContext(nc))

        # ---- persistent tiles ----
        singles = ctx.enter_context(tc.tile_pool(name="singles", bufs=1))
        qT = [singles.tile([128, L], BF16, tag=f"qT{p}", name=f"qT{p}") for p in range(NPAIR)]
        kT = [singles.tile([128, L], BF16, tag=f"kT{p}", name=f"kT{p}") for p in range(NPAIR)]
        vaug = [singles.tile([128, NH, DH + 1], BF16, tag=f"vaug{t}", name=f"vaug{t}") for t in range(T16)]
        ctxT = [singles.tile([128, L], BF16, tag=f"ctxT{p}", name=f"ctxT{p}") for p in range(NPAIR)]
        mc_sb = singles.tile([128, T16, NH], BF16, tag="mc")
        tri_sb = singles.tile([128, 128], F32, tag="tri")
        ones_sb = singles.tile([128, DH], mybir.dt.float16, tag="ones")

        nc.sync.dma_start(out=mc_sb, in_=mcol.rearrange("(t p) h -> p t h", p=128))
        nc.sync.dma_start(out=tri_sb, in_=trim)
        nc.vector.memset(ones_sb, 1.0)

        # =========== Phase 1: projections ===========
        with (
            tc.tile_pool(name="xt", bufs=10) as xtp,
            tc.tile_pool(name="w", bufs=3 * DCH) as wp,
            tc.tile_pool(name="pproj", bufs=3, space="PSUM") as ppp,
        ):
            def load_w(wdram):
                tiles = [wp.tile([128, HG], BF16, tag="w", name="wtile") for _ in range(DCH)]
                for d in range(DCH):
                    nc.sync.dma_start(out=tiles[d], in_=wdram[d * 128:(d + 1) * 128, :])
                return tiles

            def proj_transposed(xdram, wtiles, dst):
                # dst[m][:, t*512:+512] = (W.T chunk m).T @ xT  -> [outdim(P), tok(F)]
                for t in range(TQ):
                    xts = [xtp.tile([128, 512], BF16, tag="xt", name="xtile") for _ in range(DCH)]
                    for d in range(DCH):
                        nc.sync.dma_start(
                            out=xts[d],
                            in_=xdram[d * 128:(d + 1) * 128, t * 512:(t + 1) * 512],
                        )
                    for m in range(NPAIR):
                        ps = ppp.tile([128, 512], F32, tag="pproj")
                        for d in range(DCH):
                            nc.tensor.matmul(
                                ps,
                                lhsT=(wtiles[d][:, m * 128:(m + 1) * 128]),
                                rhs=(xts[d]),
                                start=(d == 0),
                                stop=(d == DCH - 1),
                            )
                        nc.vector.tensor_copy(dst[m][:, t * 512:(t + 1) * 512], ps)

            def proj_natural_v(xdram, wtiles):
                # v_aug[t16][:, h, 0:64] = (x @ Wv.T)[tok chunk, head h]
                for t in range(TQ):
                    xts = [xtp.tile([128, 512], BF16, tag="xt", name="xtile") for _ in range(DCH)]
                    for d in range(DCH):
                        nc.sync.dma_start(
                            out=xts[d],
                            in_=xdram[d * 128:(d + 1) * 128, t * 512:(t + 1) * 512],
                        )
                    for s in range(4):  # 128-token subchunks
                        t16 = t * 4 + s
                        ps = ppp.tile([128, 512], F32, tag="pproj")
                        for d in range(DCH):
                            nc.tensor.matmul(
                                ps,
                                lhsT=(xts[d][:, s * 128:(s + 1) * 128]),
                                rhs=(wtiles[d]),
                                start=(d == 0),
                                stop=(d == DCH - 1),
                            )
                        nc.vector.tensor_copy(
                            vaug[t16][:, :, 0:DH],
                            ps.rearrange("p (h e) -> p h e", h=NH),
                        )
                        nc.vector.tensor_copy(
                            vaug[t16][:, :, DH:DH + 1],
                            mc_sb[:, t16:t16 + 1, :],
                        )

            proj_transposed(xk, load_w(wk), kT)
            proj_natural_v(xv, load_w(wv))
            proj_transposed(xq, load_w(wq), qT)

        # =========== Phase 2: attention ===========
        with (
            tc.tile_pool(name="stp", bufs=2, space="PSUM") as stp,
            tc.tile_pool(name="pvp", bufs=3, space="PSUM") as pvp,
            tc.tile_pool(name="bcp", bufs=1, space="PSUM") as bcp,
            tc.tile_pool(name="expp", bufs=4) as expp,
            tc.tile_pool(name="recp", bufs=2) as recp,
            tc.tile_pool(name="bcsb", bufs=2) as bcsbp,
            tc.tile_pool(name="tmpb", bufs=2) as tmpbp,
        ):
            for p in range(NPAIR):
                for qb in range(TQ):
                    nkc = 4 * (qb + 1)  # causal k-chunks for this q-block
                    pv = [pvp.tile([DH + 1, 512], F32, tag="pv", name="pvtile") for _ in range(2)]
                    for kc in range(nkc):
                        j = kc - 4 * qb  # >=0 -> diagonal 512-block
                        off = j * 128 if j >= 0 else 0
                        st = stp.tile([128, 2, 512], F32, tag="st")
                        for ph in range(2):
                            nc.tensor.matmul(
                                st[:, ph, off:512],
                                lhsT=(kT[p][ph * DH:(ph + 1) * DH,
                                              kc * 128:(kc + 1) * 128]),
                                rhs=(qT[p][ph * DH:(ph + 1) * DH,
                                             qb * 512 + off:(qb + 1) * 512]),
                                start=True,
                                stop=True,
                            )
                        if j >= 0:  # causal mask on the first 128 q-columns
                            for ph in range(2):
                                nc.vector.tensor_add(
                                    st[:, ph, off:off + 128],
                                    st[:, ph, off:off + 128],
                                    tri_sb,
                                )
                        ex = expp.tile([128, 2, 512], BF16, tag="expst")
                        nc.scalar.activation(
                            out=ex[:, :, off:512], in_=st[:, :, off:512], func=EXP
                        )
                        for ph in range(2):
                            nc.tensor.matmul(
                                pv[ph][:, off:512],
                                lhsT=(vaug[kc][:, 2 * p + ph, :]),
                                rhs=(ex[:, ph, off:512]),
                                start=(kc == 0),
                                stop=(kc == nkc - 1),
                            )
                    # normalize: ctxT[p][:, qb block] = pv[0:64] / pv[64]
                    # sum row -> fp16 -> broadcast to 64 partitions via fp16
                    # outer product with ones -> 64-lane fast reciprocal
                    for ph in range(2):
                        rec = recp.tile([128, 512], mybir.dt.float16, tag="rec")
                        nc.vector.tensor_copy(rec[DH:DH + 1, :], pv[ph][DH:DH + 1, :])
                        bc = bcp.tile([DH, 512], F32, tag="bc")
                        nc.tensor.matmul(
                            bc,
                            lhsT=(ones_sb[DH:DH + 1, :]),
                            rhs=(rec[DH:DH + 1, :]),
                            start=True,
                            stop=True,
                        )
                        bcs = bcsbp.tile([DH, 512], F32, tag="bcs")
                        nc.vector.reciprocal_approx_fast(out=bcs, in_=bc)
                        if ph == 0:
                            nc.vector.tensor_mul(
                                ctxT[p][0:DH, qb * 512:(qb + 1) * 512],
                                pv[ph][0:DH, :],
                                bcs,
                            )
                        else:
                            tb = tmpbp.tile([DH, 512], BF16, tag="tb")
                            nc.vector.tensor_mul(tb, pv[ph][0:DH, :], bcs)
                            # partition shift rows 0-63 -> 64-127 via DMA
                            nc.sync.dma_start(
                                out=ctxT[p][DH:128, qb * 512:(qb + 1) * 512], in_=tb
                            )

        # =========== Phase 3: output projection (partial) ===========
        with (
            tc.tile_pool(name="wop", bufs=NPAIR) as wop,
            tc.tile_pool(name="outp_sb", bufs=3) as outsb,
            tc.tile_pool(name="po", bufs=3, space="PSUM") as pop,
        ):
            wo_sb = [wop.tile([128, D], BF16, tag="wo", name="wotile") for _ in range(NPAIR)]
            for c in range(NPAIR):
                nc.sync.dma_start(out=wo_sb[c], in_=wo[c * 128:(c + 1) * 128, :])
            for t16 in range(T16):
                ot = outsb.tile([128, D], F32, tag="ot")
                for oh in range(2):
                    ps = pop.tile([128, 512], F32, tag="po")
                    for c in range(NPAIR):
                        nc.tensor.matmul(
                            ps,
                            lhsT=(ctxT[c][:, t16 * 128:(t16 + 1) * 128]),
                            rhs=(wo_sb[c][:, oh * 512:(oh + 1) * 512]),
                            start=(c == 0),
                            stop=(c == NPAIR - 1),
                        )
                    nc.vector.tensor_copy(ot[:, oh * 512:(oh + 1) * 512], ps)
                nc.sync.dma_start(out=outp[t16 * 128:(t16 + 1) * 128, :], in_=ot)

    nc.compile()
    return nc


_CACHE = {}


def _get_nc(L):
    if L not in _CACHE:
        _CACHE[L] = build(L)
    return _CACHE[L]


def make_in_maps(query, key, value, attention_mask, Wq, Wk, Wv, Wo):
    import ml_dtypes

    B, L, _ = query.shape
    scale = np.float32(1.0 / math.sqrt(DH))
    bf = lambda a: np.ascontiguousarray(np.asarray(a, np.float32)).astype(
        ml_dtypes.bfloat16
    )
    xqT = [bf(np.asarray(query[b]).T) for b in range(B)]
    xkT = [bf(np.asarray(key[b]).T) for b in range(B)]
    xvT = [bf(np.asarray(value[b]).T) for b in range(B)]
    kk, qq = np.meshgrid(np.arange(128), np.arange(128), indexing="ij")
    tri = np.ascontiguousarray(
        np.where(kk > qq, np.float32(NEG), np.float32(0.0)).astype(np.float32)
    )
    in_maps = []
    for core in range(2 * B):
        b, hg = divmod(core, 2)
        sl = slice(hg * HG, (hg + 1) * HG)
        mc = np.repeat(np.asarray(attention_mask[b]).astype(np.float32)[:, None], NH, 1)
        in_maps.append({
            "xq_t": xqT[b],
            "xk_t": xkT[b],
            "xv_t": xvT[b],
            "wq_t": bf(np.asarray(Wq, np.float32)[sl, :].T * scale),
            "wk_t": bf(np.asarray(Wk, np.float32)[sl, :].T),
            "wv_t": bf(np.asarray(Wv, np.float32)[sl, :].T),
            "wo_t": bf(np.asarray(Wo, np.float32)[:, sl].T),
            "maskcol": bf(mc),
            "trimask": tri,
        })
    return in_maps


def kernel(query, key, value, attention_mask, Wq, Wk, Wv, Wo, _res_hook=None):
    B, L, D_ = query.shape
    nc = _get_nc(L)
    in_maps = make_in_maps(query, key, value, attention_mask, Wq, Wk, Wv, Wo)
    res = bass_utils.run_bass_kernel_spmd(nc, in_maps, core_ids=list(range(8)))
    if _res_hook is not None:
        _res_hook(res)
    out = np.empty((B, L, D_), np.float32)
    for b in range(B):
        out[b] = res.results[2 * b]["outp"] + res.results[2 * b + 1]["outp"]
    return out
